# revision 2
# baseline (speedup 1.0000x reference)
"""MoE expert-FFN (nn_Experts) Trainium2 kernel.

Expert-parallel: one expert per NeuronCore (E = 8 = n_cores). Host does
the dispatch (gather + dedup of duplicate routed tokens, folding the
duplicate count into the combine weight w' = k*w) and the combine
(weighted scatter-add over unique token ids). Matmul operands are bf16
(same PE rate as f32r on trn2 - 1 moving column/cycle - but half the
HBM bytes), PSUM accumulation stays fp32.

Each core runs the fused FFN for its expert over two capacity blocks of
cb = c_pad/2 unique tokens, so W1+W2 (64MB in bf16) stream from HBM
only twice (~150MB total DMA vs ~1.6ms of matmul):

    mm1: hT[f, c] = gelu(W1^T @ tokT + b1)   stationary = W1 chunk,
         moving = tok columns; gelu+bias fused in one ScalarE
         activation reading PSUM directly, writing bf16 hT in SBUF.
    mm2: outT[d, c] = (W2^T @ h) * w'[c]     stationary = W2 chunk,
         moving = hT columns (SBUF-resident); DVE applies w' from a
         partition-replicated row vector on PSUM eviction.

Output layout is [D, c_pad] (transposed); host combine untransposes.
PSUM: mm1 uses 2 sub-block accumulators x2 rotation, mm2 likewise =
exactly 8 banks. DRAM layouts give every DMA >=2KB contiguous
per-partition lines:
    tokT [P, D/P, c_pad]        (d = kc*P + p)
    w1   [F/P, P, D/P * P]      (per fb: 4KB lines)
    w2   [2*D/P, P, F/(2P) * P] (per half-db: 8KB lines)
    b1t  [P, F/P]               (f = fb*P + p)
    wvr  [P, c_pad]             (w' replicated across partitions)
"""
import numpy as np
import ml_dtypes

import concourse.bacc as bacc
import concourse.tile as tile
from concourse import mybir
from concourse.bass_utils import run_bass_kernel_spmd

P = 128
T, D, F, E, C = 8192, 2048, 8192, 8, 2048

f32 = mybir.dt.float32
bf16 = mybir.dt.bfloat16
AF = mybir.ActivationFunctionType
np_bf16 = ml_dtypes.bfloat16


def split_cblocks(c_pad):
    """Split c_pad into capacity blocks; hT for one block must fit SBUF
    (KF * cb * 2B <= ~128KB/partition => cb <= 1024)."""
    if c_pad <= 1024:
        return [c_pad]
    assert c_pad % 128 == 0
    half = (c_pad // 2 + 63) // 64 * 64
    return [half, c_pad - half]


def split_subs(cb):
    """Split a capacity block into matmul free-dim sub-blocks (<=512 for
    one fp32 PSUM bank, >=256 for full PE rate when possible)."""
    subs = []
    c0 = 0
    rem = cb
    while rem > 512:
        take = 512 if rem - 512 >= 256 or rem <= 768 else rem - 256
        if rem - take < 256 and rem > 512:
            take = rem - 256
        subs.append((c0, take))
        c0 += take
        rem -= take
    subs.append((c0, rem))
    assert sum(ns for _, ns in subs) == cb and all(ns <= 512 for _, ns in subs)
    return subs


def build_nc(c_pad):
    KD = D // P       # mm1 contraction chunks (16)
    KF = F // P       # mm2 contraction chunks (64)
    FB = F // P       # mm1 output partition groups (64)
    DB = D // P       # mm2 output partition groups (16)
    KFH = KF // 2     # w2 half-tile chunks (32)
    blocks = split_cblocks(c_pad)

    nc = bacc.Bacc()
    tokT = nc.declare_dram_parameter("tokT", [P, KD, c_pad], bf16,
                                     isOutput=False)
    w1 = nc.declare_dram_parameter("w1", [FB, P, KD * P], bf16,
                                   isOutput=False)
    w2 = nc.declare_dram_parameter("w2", [DB * 2, P, KFH * P], bf16,
                                   isOutput=False)
    b1t = nc.declare_dram_parameter("b1t", [P, FB], f32, isOutput=False)
    wvr = nc.declare_dram_parameter("wvr", [P, c_pad], f32, isOutput=False)
    out = nc.declare_dram_parameter("out", [D, c_pad], f32, isOutput=True)

    with tile.TileContext(nc) as tc:
        with tc.tile_pool(name="const", bufs=1) as const, \
             tc.tile_pool(name="tokp", bufs=1) as tokp, \
             tc.tile_pool(name="hp", bufs=1) as hp, \
             tc.tile_pool(name="w1p", bufs=2) as w1p, \
             tc.tile_pool(name="w2p", bufs=2) as w2p, \
             tc.tile_pool(name="ostp", bufs=2) as ostp, \
             tc.tile_pool(name="php", bufs=2, space="PSUM") as php, \
             tc.tile_pool(name="pop", bufs=2, space="PSUM") as pop:
            b1s = const.tile([P, FB], f32)
            nc.sync.dma_start(b1s[:], b1t[:])
            wvs = const.tile([P, c_pad], f32)
            nc.sync.dma_start(wvs[:], wvr[:])

            c_off = 0
            for cb in blocks:
                subs = split_subs(cb)
                tok_c = tokp.tile([P, KD, cb], bf16, tag="tok",
                                  name=f"tok{c_off}")
                nc.sync.dma_start(tok_c[:], tokT[:, :, c_off:c_off + cb])
                hT = hp.tile([P, KF, cb], bf16, tag="hT", name=f"hT{c_off}")

                # mm1: hT[f, :] = gelu(W1^T @ tokT + b1)
                for fb in range(FB):
                    w1t = w1p.tile([P, KD * P], bf16, tag="w1t", name="w1t")
                    nc.sync.dma_start(w1t[:], w1[fb])
                    phs = [php.tile([P, ns], f32, tag=f"ph{s}", name=f"ph{s}")
                           for s, (_, ns) in enumerate(subs)]
                    for kc in range(KD):
                        lw = w1t[:, kc * P:(kc + 1) * P]
                        for s, (c0, ns) in enumerate(subs):
                            nc.tensor.matmul(phs[s][:], lw,
                                             tok_c[:, kc, c0:c0 + ns],
                                             start=(kc == 0),
                                             stop=(kc == KD - 1))
                    for s, (c0, ns) in enumerate(subs):
                        nc.scalar.activation(hT[:, fb, c0:c0 + ns], phs[s][:],
                                             AF.Gelu_apprx_tanh,
                                             bias=b1s[:, fb:fb + 1])

                # mm2: outT[d, c] = (W2^T @ h) * w'
                for db in range(DB):
                    pos = [pop.tile([P, ns], f32, tag=f"po{s}", name=f"po{s}")
                           for s, (_, ns) in enumerate(subs)]
                    for half in range(2):
                        w2t = w2p.tile([P, KFH * P], bf16, tag="w2t",
                                       name="w2t")
                        nc.sync.dma_start(w2t[:], w2[db * 2 + half])
                        for kfl in range(KFH):
                            kf = half * KFH + kfl
                            lw = w2t[:, kfl * P:(kfl + 1) * P]
                            for s, (c0, ns) in enumerate(subs):
                                nc.tensor.matmul(pos[s][:], lw,
                                                 hT[:, kf, c0:c0 + ns],
                                                 start=(kf == 0),
                                                 stop=(kf == KF - 1))
                    ost = ostp.tile([P, cb], f32, tag="ost", name="ost")
                    for s, (c0, ns) in enumerate(subs):
                        nc.vector.tensor_tensor(
                            ost[:, c0:c0 + ns], pos[s][:],
                            wvs[:, c_off + c0:c_off + c0 + ns],
                            mybir.AluOpType.mult)
                    nc.sync.dma_start(out[db * P:(db + 1) * P,
                                          c_off:c_off + cb], ost[:])
                c_off += cb
    nc.compile()
    return nc


def pack_core(inputs, inputs_weight, top_idx, W1, b1, e, c_pad):
    """Host-side dispatch for expert e: dedup duplicate routed tokens,
    fold duplicate counts into the combine weight, gather + relayout,
    pad to c_pad rows (pad weight = 0)."""
    KD = D // P
    FB = F // P
    idx = np.asarray(top_idx[:, e])
    u, counts = np.unique(idx, return_counts=True)
    n_u = len(u)
    assert n_u <= c_pad
    w_fold = np.zeros(c_pad, dtype=np.float32)
    w_fold[:n_u] = inputs_weight[u, e].astype(np.float32) * counts
    u_pad = np.zeros(c_pad, dtype=idx.dtype)
    u_pad[:n_u] = u
    tok = np.zeros((c_pad, D), dtype=np.float32)
    tok[:n_u] = inputs[u]
    # tokT[p, kc, c] = tok[c, kc*P + p]
    tokT = tok.T.reshape(KD, P, c_pad).transpose(1, 0, 2)
    # w1m[fb, p, kc*P + j] = W1[kc*P + p, fb*P + j]
    w1m = W1[e].reshape(KD, P, FB, P).transpose(2, 1, 0, 3).reshape(
        FB, P, KD * P)
    b1m = np.ascontiguousarray(b1[e]).reshape(FB, P).T
    wvm = np.broadcast_to(w_fold[None, :], (P, c_pad))
    return u_pad, n_u, w_fold, tokT, w1m, b1m, wvm


_NC_CACHE = {}


def get_nc(c_pad):
    key = (D, F, c_pad)
    if key not in _NC_CACHE:
        _NC_CACHE[key] = build_nc(c_pad)
    return _NC_CACHE[key]


def make_in_maps(inputs, inputs_weight, top_idx, W1, b1, W2, b2):
    KF = F // P
    DB = D // P
    KFH = KF // 2
    # uniform SPMD program: pad every expert to the max unique count
    n_us = [len(np.unique(np.asarray(top_idx[:, e]))) for e in range(E)]
    c_pad = min(C, -(-max(max(n_us), 256) // P) * P)
    in_maps = []
    idxs = []
    wvs = []
    for e in range(E):
        u_pad, n_u, w_fold, tokT, w1m, b1m, wvm = pack_core(
            inputs, inputs_weight, top_idx, W1, b1, e, c_pad)
        # w2m[db*2+half, p, kfl*P + j] = W2[(half*KFH+kfl)*P + p, db*P + j]
        w2m = W2[e].reshape(KF, P, DB, P).transpose(2, 0, 1, 3).reshape(
            DB, 2, KFH, P, P).transpose(0, 1, 3, 2, 4).reshape(
            DB * 2, P, KFH * P)
        in_maps.append({
            "tokT": np.ascontiguousarray(tokT).astype(np_bf16),
            "w1": np.ascontiguousarray(w1m).astype(np_bf16),
            "w2": np.ascontiguousarray(w2m).astype(np_bf16),
            "b1t": np.ascontiguousarray(b1m, dtype=np.float32),
            "wvr": np.ascontiguousarray(wvm, dtype=np.float32),
        })
        idxs.append(u_pad)
        wvs.append(w_fold)
    return c_pad, in_maps, idxs, wvs


def combine(outs, idxs, wvs, b2):
    """Host-side combine: weighted scatter-add back to token positions.
    Device rows already carry w' = dup_count * weight; pad rows have
    w'=0. Device output is [D, c_pad] (transposed)."""
    vals = []
    for e in range(E):
        v = np.ascontiguousarray(outs[e].T)
        if np.any(b2[e]):
            v = v + wvs[e][:, None] * b2[e][None, :].astype(np.float32)
        vals.append(v)
    vals = np.concatenate(vals, axis=0)          # [E*c_pad, D]
    idx_all = np.concatenate(idxs, axis=0)       # [E*c_pad]

    order = np.argsort(idx_all, kind="stable")
    si = idx_all[order]
    sv = vals[order]
    starts = np.flatnonzero(np.r_[True, si[1:] != si[:-1]])
    sums = np.add.reduceat(sv, starts, axis=0)
    res = np.zeros((T, D), dtype=np.float32)
    res[si[starts]] = sums
    return res


def kernel(inputs, inputs_weight, top_idx, W1, b1, W2, b2):
    inputs = np.asarray(inputs, dtype=np.float32)
    inputs_weight = np.asarray(inputs_weight, dtype=np.float32)
    top_idx = np.asarray(top_idx)
    W1 = np.asarray(W1, dtype=np.float32)
    b1 = np.asarray(b1, dtype=np.float32)
    W2 = np.asarray(W2, dtype=np.float32)
    b2 = np.asarray(b2, dtype=np.float32)

    c_pad, in_maps, idxs, wvs = make_in_maps(
        inputs, inputs_weight, top_idx, W1, b1, W2, b2)
    nc = get_nc(c_pad)
    try:
        r = run_bass_kernel_spmd(nc, in_maps, list(range(E)))
    except Exception:
        # transient NRT/device hiccups happen; one retry is usually enough
        import time as _time
        _time.sleep(5)
        r = run_bass_kernel_spmd(nc, in_maps, list(range(E)))
    outs = [r.results[e]["out"] for e in range(E)]
    return combine(outs, idxs, wvs, b2)
